# revision 8
# baseline (speedup 1.0000x reference)
"""Block-sparse self-attention (DeepSpeed "fixed" layout) on 8 trn2 cores.

Problem: B=2, H=16, S=2048, D=64 fp32. Mask (identical for every head,
since numverts=1): each 64-wide diagonal window is dense, plus every 4th
16-col block ("stripe") is attended by all queries. Per 64-row query
window the attended key set = its 64 window cols + 512 stripe cols,
overlapping by 16 -> 560 distinct keys.

Sharding: 32 (b,h) pairs -> 4 per core (batch+head parallel).

v2 design (vs v1 baseline at ~116us):
- Device computes UNNORMALIZED O'^T [65, q] (V augmented with a ones
  column so row 64 carries the softmax denominator L); the host divides
  and transposes. Removes the whole on-device normalize chain
  (vector copy + DMA hop + reciprocal + gpsimd broadcast + multiply).
- Work is software-pipelined over 16 (pair, 512-query-chunk) iterations
  with QK(it+1) emitted BEFORE PV(it), so the tensor engine always has
  runnable matmuls while the scalar engine exps chunk it. Keeping PE
  busy also lets it ramp 1.2 -> 2.4 GHz.
- exp is issued as 2x [128,1024]-col + 1x [48,512]-col ACTs per chunk
  (cost on ACT is free-size cols only).
- V is pre-laid-out on the host exactly as the SBUF stationary tiles
  (stripe [128, 4*65], window [48, 32*65]) so every DMA is a few large
  contiguous descriptors instead of 2048 x 130B gathers.

On chip per (pair, qchunk) (all matmul operands at base partition 0 --
alternating weight-load base partitions between instructions faults the
device):
  S^T[k,q] = matmul(lhsT=K^T chunk, rhs=Q^T chunk)      (PSUM fp32)
  P = exp(0.125 * S^T)  on ACT, fp16 -> SBUF            (scale fused)
  O'^T[65,512] += matmul(lhsT=V_aug chunk, rhs=P chunk) (PSUM fp32)
  DVE copy O' -> SBUF fp32, DMA to DRAM.
"""

import numpy as np

B, H, S, D = 2, 16, 2048, 64
NPAIRS = B * H
NCORES = 8
P_PER_CORE = NPAIRS // NCORES  # 4
NCH = 4        # stripe k-chunks of 128
NW = S // 64   # 32 windows
NQC = 4        # query chunks of 512 per pair
QC = S // NQC  # 512
SCALE = float(D) ** -0.5


def _reorder_idx():
    blocks = np.arange(S // 16)
    stripe = blocks[blocks % 4 == 3]
    rest = blocks[blocks % 4 != 3]
    cols = np.arange(S).reshape(-1, 16)
    return np.concatenate([cols[stripe].ravel(), cols[rest].ravel()])


_REORDER = _reorder_idx()

_CACHE = {}


def _build(dt_in_name="float16", npairs=P_PER_CORE):
    from contextlib import ExitStack
    import concourse.bacc as bacc
    import concourse.tile as tile
    from concourse import mybir

    dt_in = getattr(mybir.dt, dt_in_name)
    f32 = mybir.dt.float32
    EXP = mybir.ActivationFunctionType.Exp

    nc = bacc.Bacc("TRN2", target_bir_lowering=False, debug=False,
                   num_devices=NCORES)
    qT = nc.dram_tensor("qT", [npairs, 64, S], dt_in,
                        kind="ExternalInput").ap()
    kT = nc.dram_tensor("kT", [npairs, 64, S], dt_in,
                        kind="ExternalInput").ap()
    vsd = nc.dram_tensor("vsd", [npairs, 128, NCH * 65], dt_in,
                         kind="ExternalInput").ap()
    vwd = nc.dram_tensor("vwd", [npairs, 48, NW * 65], dt_in,
                         kind="ExternalInput").ap()
    out = nc.dram_tensor("out", [npairs, 65, S], f32,
                         kind="ExternalOutput").ap()

    NIT = npairs * NQC  # 16 pipelined iterations

    with tile.TileContext(nc) as tc, ExitStack() as ctx:
        in_pool = ctx.enter_context(tc.tile_pool(name="in", bufs=1))
        ps_pool = ctx.enter_context(tc.tile_pool(name="ps", bufs=2))
        ob_pool = ctx.enter_context(tc.tile_pool(name="ob", bufs=2))
        s_pool = ctx.enter_context(tc.tile_pool(name="s", bufs=2, space="PSUM"))
        w_pool = ctx.enter_context(tc.tile_pool(name="w", bufs=1, space="PSUM"))
        o_pool = ctx.enter_context(tc.tile_pool(name="o", bufs=2, space="PSUM"))

        # kT split into window cols (needed by the first matmul, since
        # windows go first) and stripe cols. Pair 0's Q additionally split
        # so the first iteration only waits on a 128KB chunk.
        qt = {}    # p -> [64, 2048] (pairs 1+); pair 0: q00 + q0r
        kts, ktw, vs, vw = {}, {}, {}, {}

        def load_pair(p):
            if p == 0:
                t = in_pool.tile([64, QC], dt_in, tag="q00")
                nc.sync.dma_start(out=t, in_=qT[0, :, 0:QC])
                q00 = t
                t = in_pool.tile([64, 1536], dt_in, tag="kw0")
                nc.sync.dma_start(out=t, in_=kT[0, :, 512:S])
                ktw[0] = t
                t = in_pool.tile([64, 512], dt_in, tag="ks0")
                nc.sync.dma_start(out=t, in_=kT[0, :, 0:512])
                kts[0] = t
                t = in_pool.tile([64, 3 * QC], dt_in, tag="q0r")
                nc.sync.dma_start(out=t, in_=qT[0, :, QC:S])
                qt[0] = (q00, t)
            else:
                t = in_pool.tile([64, S], dt_in, tag=f"q{p}")
                nc.sync.dma_start(out=t, in_=qT[p])
                qt[p] = t
                t = in_pool.tile([64, 1536], dt_in, tag=f"kw{p}")
                nc.sync.dma_start(out=t, in_=kT[p, :, 512:S])
                ktw[p] = t
                t = in_pool.tile([64, 512], dt_in, tag=f"ks{p}")
                nc.sync.dma_start(out=t, in_=kT[p, :, 0:512])
                kts[p] = t
            t = in_pool.tile([128, NCH * 65], dt_in, tag=f"vs{p}")
            nc.sync.dma_start(out=t, in_=vsd[p])
            vs[p] = t
            t = in_pool.tile([48, NW * 65], dt_in, tag=f"vw{p}")
            nc.sync.dma_start(out=t, in_=vwd[p])
            vw[p] = t

        for p in range(npairs):
            load_pair(p)

        def qslice(p, g):
            if p == 0:
                return qt[0][0] if g == 0 else \
                    qt[0][1][:, (g - 1) * QC:g * QC]
            return qt[p][:, g * QC:(g + 1) * QC]

        # per-iteration state carried from QK/exp stage to PV stage
        live = {}

        def emit_qk(it):
            p, g = divmod(it, NQC)
            qs = qslice(p, g)
            ps = ps_pool.tile([128, NCH * QC], dt_in, tag="ps")
            # windows FIRST: the single-buffered sw tile's round trip
            # (QKwin -> ACTwin -> next QKwin) then has a full iteration of
            # slack instead of gating the pipeline
            sw = w_pool.tile([48, QC], f32, tag="w")
            for wi in range(8):
                nc.tensor.matmul(
                    out=sw[:, wi * 64:(wi + 1) * 64],
                    lhsT=ktw[p][:, 48 * (8 * g + wi):48 * (8 * g + wi) + 48],
                    rhs=qs[:, 64 * wi:64 * wi + 64],
                    start=True, stop=True)
            pw = ps_pool.tile([48, QC], dt_in, tag="pw")
            nc.scalar.activation(out=pw, in_=sw, func=EXP, scale=SCALE)
            # stripe scores: two [128,1024] PSUM tiles, each = 2 k-chunks
            for hf in range(2):
                st = s_pool.tile([128, 1024], f32, tag="s")
                for j in range(2):
                    c = 2 * hf + j
                    nc.tensor.matmul(
                        out=st[:, j * QC:(j + 1) * QC],
                        lhsT=kts[p][:, c * 128:(c + 1) * 128],
                        rhs=qs,
                        start=True, stop=True)
                nc.scalar.activation(
                    out=ps[:, hf * 1024:(hf + 1) * 1024], in_=st,
                    func=EXP, scale=SCALE)
            live[it] = (ps, pw)

        def emit_pv(it):
            p, g = divmod(it, NQC)
            q0 = g * QC
            ps, pw = live.pop(it)
            ov = o_pool.tile([65, QC], f32, tag="o")
            for c in range(NCH):
                nc.tensor.matmul(
                    out=ov,
                    lhsT=vs[p][:, c * 65:(c + 1) * 65],
                    rhs=ps[:, c * QC:(c + 1) * QC],
                    start=(c == 0), stop=False, skip_group_check=True)
            for wi in range(8):
                w = g * 8 + wi
                nc.tensor.matmul(
                    out=ov[:, wi * 64:(wi + 1) * 64],
                    lhsT=vw[p][:, 65 * w:65 * w + 65],
                    rhs=pw[:, wi * 64:(wi + 1) * 64],
                    start=False, stop=(wi == 7), skip_group_check=True)
            ob = ob_pool.tile([65, QC], f32, tag="ob")
            nc.vector.tensor_copy(ob, ov)
            # out-DMA triggers ride the (otherwise idle) gpsimd queue so
            # they don't serialize behind input triggers on sync
            nc.gpsimd.dma_start(out=out[p, :, q0:q0 + QC], in_=ob)

        # software pipeline: QK(it+1) ahead of PV(it) so the tensor queue
        # always has work while ACT exps chunk it
        emit_qk(0)
        for it in range(1, NIT):
            emit_qk(it)
            emit_pv(it - 1)
        emit_pv(NIT - 1)

    nc.compile()
    return nc


def _get_nc(dt_in_name="float16"):
    if dt_in_name not in _CACHE:
        _CACHE[dt_in_name] = _build(dt_in_name)
    return _CACHE[dt_in_name]


def _prep_inputs(query, key, value, np_dt):
    q = np.asarray(query).reshape(NPAIRS, S, D)
    k = np.asarray(key).reshape(NPAIRS, S, D)
    v = np.asarray(value).reshape(NPAIRS, S, D)
    kr = k[:, _REORDER, :]
    vr = v[:, _REORDER, :]
    qT = np.ascontiguousarray(q.transpose(0, 2, 1)).astype(np_dt)
    kT = np.ascontiguousarray(kr.transpose(0, 2, 1)).astype(np_dt)
    va = np.concatenate(
        [vr, np.ones((NPAIRS, S, 1), vr.dtype)], axis=2).astype(np_dt)
    # stripe V in stationary-tile layout [128, 4*65]
    vsd = np.ascontiguousarray(
        va[:, 0:512].reshape(NPAIRS, NCH, 128, 65).transpose(0, 2, 1, 3)
        .reshape(NPAIRS, 128, NCH * 65))
    # window V in stationary-tile layout [48, 32*65]
    vwd = np.ascontiguousarray(
        va[:, 512:S].reshape(NPAIRS, NW, 48, 65).transpose(0, 2, 1, 3)
        .reshape(NPAIRS, 48, NW * 65))
    in_maps = []
    for core in range(NCORES):
        sl = slice(core * P_PER_CORE, (core + 1) * P_PER_CORE)
        in_maps.append({"qT": np.ascontiguousarray(qT[sl]),
                        "kT": np.ascontiguousarray(kT[sl]),
                        "vsd": np.ascontiguousarray(vsd[sl]),
                        "vwd": np.ascontiguousarray(vwd[sl])})
    return in_maps


def _run(query, key, value, dt_in_name="float16", trace=False):
    from concourse.bass_utils import run_bass_kernel_spmd
    nc = _get_nc(dt_in_name)
    in_maps = _prep_inputs(query, key, value, np.float16
                           if dt_in_name == "float16" else np.float32)
    res = run_bass_kernel_spmd(nc, in_maps, list(range(NCORES)), trace=trace)
    o = np.concatenate([res.results[i]["out"] for i in range(NCORES)], axis=0)
    # host-side softmax normalization: row 64 is the denominator L
    full = (o[:, :64, :] / o[:, 64:65, :]).transpose(0, 2, 1)
    full = np.ascontiguousarray(full).reshape(B, H, S, D).astype(np.float32)
    return full, res


def kernel(query, key, value):
    full, _ = _run(np.asarray(query), np.asarray(key), np.asarray(value))
    return full


# revision 14
# speedup vs baseline: 1.2931x; 1.2931x over previous
"""Block-sparse self-attention (DeepSpeed "fixed" layout) on 8 trn2 cores.

Problem: B=2, H=16, S=2048, D=64 fp32. Mask (identical for every head,
since numverts=1): each 64-wide diagonal window is dense, plus every 4th
16-col block ("stripe") is attended by all queries. Per 64-row query
window the attended key set = its 64 window cols + 512 stripe cols,
overlapping by 16 -> 560 distinct keys.

Sharding: 32 (b,h) pairs -> 4 per core (batch+head parallel).

v2 design (vs v1 baseline at ~116us):
- Device computes UNNORMALIZED O'^T [65, q] (V augmented with a ones
  column so row 64 carries the softmax denominator L); the host divides
  and transposes. Removes the whole on-device normalize chain
  (vector copy + DMA hop + reciprocal + gpsimd broadcast + multiply).
- Work is software-pipelined over 16 (pair, 512-query-chunk) iterations
  with QK(it+1) emitted BEFORE PV(it), so the tensor engine always has
  runnable matmuls while the scalar engine exps chunk it. Keeping PE
  busy also lets it ramp 1.2 -> 2.4 GHz.
- exp is issued as 2x [128,1024]-col + 1x [48,512]-col ACTs per chunk
  (cost on ACT is free-size cols only).
- V is pre-laid-out on the host exactly as the SBUF stationary tiles
  (stripe [128, 4*65], window [48, 32*65]) so every DMA is a few large
  contiguous descriptors instead of 2048 x 130B gathers.

On chip per (pair, qchunk) (all matmul operands at base partition 0 --
alternating weight-load base partitions between instructions faults the
device):
  S^T[k,q] = matmul(lhsT=K^T chunk, rhs=Q^T chunk)      (PSUM fp32)
  P = exp(0.125 * S^T)  on ACT, fp16 -> SBUF            (scale fused)
  O'^T[65,512] += matmul(lhsT=V_aug chunk, rhs=P chunk) (PSUM fp32)
  DVE copy O' -> SBUF fp32, DMA to DRAM.
"""

import numpy as np

B, H, S, D = 2, 16, 2048, 64
NPAIRS = B * H
NCORES = 8
P_PER_CORE = NPAIRS // NCORES  # 4
NCH = 4        # stripe k-chunks of 128
NW = S // 64   # 32 windows
NQC = 4        # query chunks of 512 per pair
QC = S // NQC  # 512
SCALE = float(D) ** -0.5


def _reorder_idx():
    blocks = np.arange(S // 16)
    stripe = blocks[blocks % 4 == 3]
    rest = blocks[blocks % 4 != 3]
    cols = np.arange(S).reshape(-1, 16)
    return np.concatenate([cols[stripe].ravel(), cols[rest].ravel()])


_REORDER = _reorder_idx()

_CACHE = {}


def _build(dt_in_name="float16", npairs=P_PER_CORE):
    from contextlib import ExitStack
    import concourse.bacc as bacc
    import concourse.tile as tile
    from concourse import mybir

    dt_in = getattr(mybir.dt, dt_in_name)
    f32 = mybir.dt.float32
    EXP = mybir.ActivationFunctionType.Exp

    nc = bacc.Bacc("TRN2", target_bir_lowering=False, debug=False,
                   num_devices=NCORES)
    qT = nc.dram_tensor("qT", [npairs, 64, S], dt_in,
                        kind="ExternalInput").ap()
    kT = nc.dram_tensor("kT", [npairs, 64, S], dt_in,
                        kind="ExternalInput").ap()
    vsd = nc.dram_tensor("vsd", [npairs, 128, NCH * 65], dt_in,
                         kind="ExternalInput").ap()
    vwd = nc.dram_tensor("vwd", [npairs, 48, NW * 65], dt_in,
                         kind="ExternalInput").ap()
    out = nc.dram_tensor("out", [npairs, 65, S], f32,
                         kind="ExternalOutput").ap()

    NIT = npairs * NQC  # 16 pipelined iterations

    with tile.TileContext(nc) as tc, ExitStack() as ctx:
        in_pool = ctx.enter_context(tc.tile_pool(name="in", bufs=1))
        ps_pool = ctx.enter_context(tc.tile_pool(name="ps", bufs=2))
        ob_pool = ctx.enter_context(tc.tile_pool(name="ob", bufs=2))
        s_pool = ctx.enter_context(tc.tile_pool(name="s", bufs=2, space="PSUM"))
        w_pool = ctx.enter_context(tc.tile_pool(name="w", bufs=1, space="PSUM"))
        o_pool = ctx.enter_context(tc.tile_pool(name="o", bufs=2, space="PSUM"))

        # resident inputs for all pairs (SBUF is big enough); DMAs all
        # start immediately and overlap with compute. Pair 0's q and kt
        # are split so iteration 0 starts after ~3 small transfers.
        qt, kt, vs, vw = [], [], [], []
        for p in range(npairs):
            if p == 0:
                q00 = in_pool.tile([64, QC], dt_in, tag="q00")
                nc.sync.dma_start(out=q00, in_=qT[0, :, 0:QC])
                # split at col 992 = start of window 10, so no window's
                # 48 cols straddle the two tiles
                k0a = in_pool.tile([64, 992], dt_in, tag="k0a")
                nc.sync.dma_start(out=k0a, in_=kT[0, :, 0:992])
                k0b = in_pool.tile([64, S - 992], dt_in, tag="k0b")
                nc.sync.dma_start(out=k0b, in_=kT[0, :, 992:S])
                q0r = in_pool.tile([64, 3 * QC], dt_in, tag="q0r")
                nc.sync.dma_start(out=q0r, in_=qT[0, :, QC:S])
                qt.append((q00, q0r))
                kt.append((k0a, k0b))
            else:
                t = in_pool.tile([64, S], dt_in, tag=f"q{p}")
                nc.sync.dma_start(out=t, in_=qT[p])
                qt.append(t)
                t = in_pool.tile([64, S], dt_in, tag=f"k{p}")
                nc.sync.dma_start(out=t, in_=kT[p])
                kt.append(t)
            t = in_pool.tile([128, NCH * 65], dt_in, tag=f"vs{p}")
            nc.sync.dma_start(out=t, in_=vsd[p])
            vs.append(t)
            t = in_pool.tile([48, NW * 65], dt_in, tag=f"vw{p}")
            nc.sync.dma_start(out=t, in_=vwd[p])
            vw.append(t)

        def q_ap(p, g, lo, hi):
            # query cols [g*QC+lo, g*QC+hi) of pair p
            if p == 0:
                if g == 0:
                    return qt[0][0][:, lo:hi]
                base = (g - 1) * QC
                return qt[0][1][:, base + lo:base + hi]
            return qt[p][:, g * QC + lo:g * QC + hi]

        def k_ap(p, lo, hi):
            # key cols [lo, hi) of pair p (within reordered kT)
            if p == 0:
                if hi <= 992:
                    return kt[0][0][:, lo:hi]
                return kt[0][1][:, lo - 992:hi - 992]
            return kt[p][:, lo:hi]

        # per-iteration state carried from QK/exp stage to PV stage
        live = {}

        def emit_qk(it):
            p, g = divmod(it, NQC)
            q0 = g * QC
            ps = ps_pool.tile([128, NCH * QC], dt_in, tag="ps")
            # stripe scores: two [128,1024] PSUM tiles, each = 2 k-chunks
            for hf in range(2):
                st = s_pool.tile([128, 1024], f32, tag="s")
                for j in range(2):
                    c = 2 * hf + j
                    nc.tensor.matmul(
                        out=st[:, j * QC:(j + 1) * QC],
                        lhsT=k_ap(p, c * 128, (c + 1) * 128),
                        rhs=q_ap(p, g, 0, QC),
                        start=True, stop=True)
                nc.scalar.activation(
                    out=ps[:, hf * 1024:(hf + 1) * 1024], in_=st,
                    func=EXP, scale=SCALE)
            # window scores for the 8 windows of this q chunk
            sw = w_pool.tile([48, QC], f32, tag="w")
            for wi in range(8):
                w = g * 8 + wi
                nc.tensor.matmul(
                    out=sw[:, wi * 64:(wi + 1) * 64],
                    lhsT=k_ap(p, 512 + 48 * w, 512 + 48 * w + 48),
                    rhs=q_ap(p, g, 64 * wi, 64 * wi + 64),
                    start=True, stop=True)
            pw = ps_pool.tile([48, QC], dt_in, tag="pw")
            nc.scalar.activation(out=pw, in_=sw, func=EXP, scale=SCALE)
            live[it] = (ps, pw)

        def emit_pv(it):
            p, g = divmod(it, NQC)
            q0 = g * QC
            ps, pw = live.pop(it)
            ov = o_pool.tile([65, QC], f32, tag="o")
            for c in range(NCH):
                nc.tensor.matmul(
                    out=ov,
                    lhsT=vs[p][:, c * 65:(c + 1) * 65],
                    rhs=ps[:, c * QC:(c + 1) * QC],
                    start=(c == 0), stop=False, skip_group_check=True)
            for wi in range(8):
                w = g * 8 + wi
                nc.tensor.matmul(
                    out=ov[:, wi * 64:(wi + 1) * 64],
                    lhsT=vw[p][:, 65 * w:65 * w + 65],
                    rhs=pw[:, wi * 64:(wi + 1) * 64],
                    start=False, stop=(wi == 7), skip_group_check=True)
            ob = ob_pool.tile([65, QC], f32, tag="ob")
            nc.vector.tensor_copy(ob, ov)
            nc.sync.dma_start(out=out[p, :, q0:q0 + QC], in_=ob)

        # software pipeline: QK(it+1) ahead of PV(it) so the tensor queue
        # always has work while ACT exps chunk it
        emit_qk(0)
        for it in range(1, NIT):
            emit_qk(it)
            emit_pv(it - 1)
        emit_pv(NIT - 1)

    nc.compile()
    return nc


def _get_nc(dt_in_name="float16"):
    if dt_in_name not in _CACHE:
        _CACHE[dt_in_name] = _build(dt_in_name)
    return _CACHE[dt_in_name]


def _prep_inputs(query, key, value, np_dt):
    q = np.asarray(query).reshape(NPAIRS, S, D)
    k = np.asarray(key).reshape(NPAIRS, S, D)
    v = np.asarray(value).reshape(NPAIRS, S, D)
    kr = k[:, _REORDER, :]
    vr = v[:, _REORDER, :]
    qT = np.ascontiguousarray(q.transpose(0, 2, 1)).astype(np_dt)
    kT = np.ascontiguousarray(kr.transpose(0, 2, 1)).astype(np_dt)
    va = np.concatenate(
        [vr, np.ones((NPAIRS, S, 1), vr.dtype)], axis=2).astype(np_dt)
    # stripe V in stationary-tile layout [128, 4*65]
    vsd = np.ascontiguousarray(
        va[:, 0:512].reshape(NPAIRS, NCH, 128, 65).transpose(0, 2, 1, 3)
        .reshape(NPAIRS, 128, NCH * 65))
    # window V in stationary-tile layout [48, 32*65]
    vwd = np.ascontiguousarray(
        va[:, 512:S].reshape(NPAIRS, NW, 48, 65).transpose(0, 2, 1, 3)
        .reshape(NPAIRS, 48, NW * 65))
    in_maps = []
    for core in range(NCORES):
        sl = slice(core * P_PER_CORE, (core + 1) * P_PER_CORE)
        in_maps.append({"qT": np.ascontiguousarray(qT[sl]),
                        "kT": np.ascontiguousarray(kT[sl]),
                        "vsd": np.ascontiguousarray(vsd[sl]),
                        "vwd": np.ascontiguousarray(vwd[sl])})
    return in_maps


def _run(query, key, value, dt_in_name="float16", trace=False):
    from concourse.bass_utils import run_bass_kernel_spmd
    nc = _get_nc(dt_in_name)
    in_maps = _prep_inputs(query, key, value, np.float16
                           if dt_in_name == "float16" else np.float32)
    res = run_bass_kernel_spmd(nc, in_maps, list(range(NCORES)), trace=trace)
    o = np.concatenate([res.results[i]["out"] for i in range(NCORES)], axis=0)
    # host-side softmax normalization: row 64 is the denominator L
    full = (o[:, :64, :] / o[:, 64:65, :]).transpose(0, 2, 1)
    full = np.ascontiguousarray(full).reshape(B, H, S, D).astype(np.float32)
    return full, res


def kernel(query, key, value):
    full, _ = _run(np.asarray(query), np.asarray(key), np.asarray(value))
    return full
